# revision 43
# baseline (speedup 1.0000x reference)
"""Trainium2 Bass kernel: softmax spatial pooling (attention pooling).

Reference computation per batch b:
    attn = softmax(probs[b].reshape(19, 16384), axis=1)
    ctx  = attn @ feats[b].reshape(512, 16384).T        # (19, 512)
    out[b] = ctx.T[..., None]                           # (512, 19, 1)

Full inputs:  feats (8, 512, 128, 128) f32, probs (8, 19, 128, 128) f32.
Sharding: pure data parallel - one batch sample per NeuronCore (8 cores).

Device-side algorithm (per core):
  View n = 16384 as (n1=128, n2=128) with n1 on SBUF partitions; the PE
  contracts over n1 (partitions) and PSUM accumulates over n2.
  Softmax is unnormalized (E = exp(p)); the 1/sum normalization is applied
  once to the small per-chunk result.

  feats (32 MB fp32, the dominant cost) loads via SWDGE cast-DMA
  (fp32->bf16 in the DMA, 512B descriptors at bus rate ~22ns/desc).  That
  forces SBUF layout [n1, c, n2].  Matmul layouts:
    'fe'     feats tile is the STATIONARY operand (lhsT, strided columns),
             E is the streamed rhs (contiguous, 19 cols); out [c, k].
    'swdge2' reorder [n1,c,n2]->[n1,n2,c] on engines, E stationary,
             feats rhs contiguous; out [k, c].
    'swdge'  no reorder, feats rhs strided (4 cyc/col PE penalty).
"""

import numpy as np

import concourse.bacc as bacc
import concourse.bass as bass
import concourse.tile as tile
from concourse import mybir
from concourse.bass_utils import run_bass_kernel_spmd

B = 8          # batch == number of cores
C = 512        # feature channels
K = 19         # attention heads (probs channels)
N1 = 128       # spatial high bits -> SBUF partitions
N2 = 128       # spatial low bits  -> PSUM-accumulated matmuls

LAYOUT = "swdge2"    # 'nat' | 'fe' | 'swdge2' | 'swdge' | 'reorder' | 'direct'
CCHUNK = 128         # matmul tile width (c channels per PSUM group)
FCHUNK = 32          # channels per feats DMA piece
RCHUNK = 32          # channels per reorder-copy op (sub-slices of a piece).
                     # 16 (half-ops on both engines concurrently, DVE ops
                     # inside the ring buffer) measured neutral: 126.4us
                     # vs 124-129 here; the extra per-op overheads eat
                     # the concurrency gain.
F32_BUFS = 10
# Every HW_EVERY-th piece (offset 2) loads fp32 via the HWDGE sync ring
# with a fused DVE reorder-cast, relieving the Q7 SWDGE descriptor
# generator.  Measured SLOWER (135us vs 124us): the pricier HWDGE
# descriptors and DVE port-lock slivers outweigh the Q7 relief.  Disabled.
HW_EVERY = 0
B16_BUFS = 2
# Reorder-copy engines ('swdge2'/'reorder'): alternate Scalar/Vector.  A DVE
# copy locks SWDGE descriptor generation out of the shared GPSIMD/Vector
# SBUF port for its duration - reorder ops are kept small (~3.6us DVE ops)
# so the descriptor ring (DMA_SCRATCH/16 descs of drain buffer) mostly
# rides out the lockout.
CAST_ENGINES = "sv"
DMA_SCRATCH = 32768  # SWDGE descriptor-ring bytes/partition (16B/desc).
                     # 49152 (ring big enough to fully cover a DVE
                     # reorder's SWDGE lockout) measured SLOWER (136us);
                     # keep 32768.

# 'nat' layout parameters
W = 2048             # n-elements per feats DMA slab (8KB descriptors)
NAT_F32_BUFS = 4
NAT_B16_BUFS = 4
NAT_FFB_BUFS = 2

F32 = mybir.dt.float32
BF16 = mybir.dt.bfloat16


def _body_nat(tc, pools, out, feats, probs):
    """Natural-layout path: HWDGE big-descriptor fp32 loads (no SWDGE at
    all), on-chip fp32->bf16 cast, PE transpose of 128x128 blocks via
    identity matmuls, PSUM evacuation into MM-ready [n_lo, s, c] layout,
    and one persistent PSUM bank accumulating the (K, C) output over all
    128 slice-matmuls of N=C columns.

    n = 128*s + j: s = high 7 bits (slice index), j = low 7 bits (partitions
    after transpose).  E must match: E-slices [j, k] per s come from 19
    PE transposes of the exp'd probs.
    """
    nc = tc.nc
    ffp32, ffp16, ffbp, eep, smallp, csbp, pscp, psep, pssp = pools
    SLABS = (N1 * N2) // W   # feats DMA slabs
    TPS = W // 128           # 128-blocks per slab
    CB = C // 128            # channel blocks

    # identity [128, 128] bf16: iota(j - p) == 0   (iota lives on gpsimd,
    # which is otherwise idle in this layout)
    iot = smallp.tile([128, 128], mybir.dt.int32, name="iot")
    nc.gpsimd.iota(iot[:], pattern=[[1, 128]], base=0, channel_multiplier=-1)
    ident = smallp.tile([128, 128], BF16, name="ident")
    nc.gpsimd.tensor_scalar(
        ident[:], iot[:], 0, None, op0=mybir.AluOpType.is_equal
    )

    # probs (K, N1*N2) -> (N1, K, N2) on the scalar HWDGE ring
    probs_r = probs.rearrange("k (n1 n2) -> n1 k n2", n1=N1)
    ee = eep.tile([N1, K, N2], F32)
    nc.scalar.dma_start(out=ee[:], in_=probs_r)

    # E = exp(p), [n1, k, n2] bf16 (contiguous src AND dst)
    eeekn = eep.tile([N1, K, N2], BF16)
    nc.scalar.activation(eeekn[:], ee[:], mybir.ActivationFunctionType.Exp)

    # partials[n1, k] = sum_n2 E[k, n1, n2] (contiguous reduce)
    partials = smallp.tile([N1, K, 1], F32)
    nc.vector.reduce_sum(out=partials[:], in_=eeekn[:], axis=mybir.AxisListType.X)
    ones = smallp.tile([N1, 1], F32)
    nc.vector.memset(ones[:], 1.0)
    s_ps = pssp.tile([K, 1], F32)
    nc.tensor.matmul(s_ps[:], partials[:, :, 0], ones[:], start=True, stop=True)
    rec = smallp.tile([K, 1], F32)
    nc.vector.reciprocal(rec[:], s_ps[:])

    # E-transpose: eee2[j, s, k] = E[k, 128*s + j] via 19 PE transposes
    eee2 = eep.tile([N2, N1, K], BF16)
    for k in range(K):
        psE = psep.tile([128, N1], F32)
        nc.tensor.matmul(psE[:], eeekn[:, k, :], ident[:], start=True, stop=True)
        nc.vector.tensor_copy(eee2[:, :, k], psE[:])

    # feats loads: [c-block 128 rows, W-column slab] fp32, 8KB descriptors
    feats_cb = feats.rearrange("(cb p) n -> cb p n", p=128)
    big_ps = pscp.tile([K, C], F32)
    cast_i = 0
    evac_i = 0
    for slab in range(SLABS):
        ffb = ffbp.tile([128, TPS, C], BF16)
        for cb in range(CB):
            f32 = ffp32.tile([128, W], F32)
            nc.sync.dma_start(
                out=f32[:], in_=feats_cb[cb, :, slab * W : (slab + 1) * W]
            )
            b16 = ffp16.tile([128, W], BF16)
            if cast_i % 2 == 0:
                nc.vector.tensor_copy(b16[:], f32[:])
            else:
                nc.scalar.copy(b16[:], f32[:])
            cast_i += 1
            # PE transposes, 4 blocks per PSUM bank, then one evacuation
            for t0 in range(0, TPS, 4):
                psT = psep.tile([128, 512], F32, tag="psT")
                for u in range(4):
                    nc.tensor.matmul(
                        psT[:, u * 128 : (u + 1) * 128],
                        b16[:, (t0 + u) * 128 : (t0 + u + 1) * 128],
                        ident[:],
                        start=True, stop=True,
                    )
                dst = ffb[:, t0 : t0 + 4, cb * 128 : (cb + 1) * 128]
                src = psT[:].rearrange("p (t c) -> p t c", t=4)
                if evac_i % 2 == 0:
                    nc.scalar.copy(dst, src)
                else:
                    nc.vector.tensor_copy(dst, src)
                evac_i += 1
        # slice-matmuls for this slab, all accumulating into big_ps
        for t in range(TPS):
            s = slab * TPS + t
            nc.tensor.matmul(
                big_ps[:], eee2[:, s, :], ffb[:, t, :],
                start=(s == 0), stop=(s == N1 - 1),
            )

    # normalize and store (K, C)
    c_sb = csbp.tile([K, C], F32)
    nc.scalar.activation(
        c_sb[:], big_ps[:], mybir.ActivationFunctionType.Copy, scale=rec[:]
    )
    nc.scalar.dma_start(out=out[:], in_=c_sb[:])


def _body(tc, pools, out, feats, probs):
    nc = tc.nc
    ffp32, ffp16, ffhp, eep, smallp, csbp, pscp, pssp = pools

    # probs (K, N1*N2) -> (N1, K, N2): 512B contiguous runs per (n1, k).
    # On the scalar (Activation) HWDGE ring so it never queues behind the
    # feats transfers.
    probs_r = probs.rearrange("k (n1 n2) -> n1 k n2", n1=N1)
    ee = eep.tile([N1, K, N2], F32)
    nc.scalar.dma_start(out=ee[:], in_=probs_r)

    # E = exp(p) stored [n1, n2, k] so each matmul slice [:, n2, :] is
    # contiguous.  The strided rearrange goes on the SOURCE side: engines
    # stream ~1 elem/cycle with a contiguous dst even when src is strided,
    # but a strided dst costs ~5 cycles/elem.
    eee = eep.tile([N1, N2, K], BF16)
    nc.scalar.activation(
        eee[:], ee[:].rearrange("p k n -> p n k"),
        mybir.ActivationFunctionType.Exp,
    )

    # partials[n1, k] = sum_n2 E[k, n1, n2]
    partials = smallp.tile([N1, K, 1], F32)
    nc.vector.reduce_sum(
        out=partials[:], in_=eee[:].rearrange("p n k -> p k n"),
        axis=mybir.AxisListType.X,
    )

    ones = smallp.tile([N1, 1], F32)
    nc.vector.memset(ones[:], 1.0)
    if LAYOUT == "fe":
        # S as a row vector (1, K); reciprocal; broadcast down 128
        # partitions via a rank-1 matmul so DVE can multiply along free dim.
        s_ps = pssp.tile([1, K], F32)
        nc.tensor.matmul(s_ps[:], ones[:], partials[:, :, 0], start=True, stop=True)
        rec_t = smallp.tile([1, K], F32)
        nc.vector.reciprocal(rec_t[:], s_ps[:])
        ones_row = smallp.tile([1, N1], F32)
        nc.vector.memset(ones_row[:], 1.0)
        bc_ps = pssp.tile([N1, K], F32, tag="bc")
        nc.tensor.matmul(bc_ps[:], ones_row[:], rec_t[:], start=True, stop=True)
        rec_b = smallp.tile([N1, K], F32)
        nc.scalar.copy(rec_b[:], bc_ps[:])
        rec = None
    else:
        # S[k] = sum_n1 partials[n1, k]; rec = 1/S per partition (k rows)
        s_ps = pssp.tile([K, 1], F32)
        nc.tensor.matmul(s_ps[:], partials[:, :, 0], ones[:], start=True, stop=True)
        rec = smallp.tile([K, 1], F32)
        nc.vector.reciprocal(rec[:], s_ps[:])
        rec_b = None

    # feats (C, N1*N2) -> (N1, C, N2) view; DMA pieces of FCHUNK channels
    feats_r = feats.rearrange("c (n1 n2) -> n1 c n2", n1=N1)
    n_f_per_c = CCHUNK // FCHUNK
    cast_i = 0

    def cast(dst_view, src_view):
        nonlocal cast_i
        eng = CAST_ENGINES[cast_i % len(CAST_ENGINES)]
        cast_i += 1
        if eng == "v":
            nc.vector.tensor_copy(dst_view, src_view)
        elif eng == "s":
            nc.scalar.copy(dst_view, src_view)
        else:
            nc.gpsimd.tensor_copy(dst_view, src_view)

    for cc in range(C // CCHUNK):
        if LAYOUT in ("reorder", "swdge2"):
            ffb = ffp16.tile([N1, N2, CCHUNK], BF16)
        else:
            ffb = ffp16.tile([N1, CCHUNK, N2], BF16)
        for fc in range(n_f_per_c):
            c0 = cc * CCHUNK + fc * FCHUNK
            lo, hi = fc * FCHUNK, (fc + 1) * FCHUNK
            src = feats_r[:, c0 : c0 + FCHUNK, :]
            if LAYOUT in ("swdge", "fe"):
                # fp32->bf16 cast inside the SWDGE DMA; one descriptor per
                # (n1, c) 512B run
                nc.gpsimd.dma_start(out=ffb[:, lo:hi, :], in_=src)
                continue
            if LAYOUT == "swdge2":
                gfc = cc * n_f_per_c + fc
                if HW_EVERY and gfc % HW_EVERY == 2:
                    # HWDGE fp32 load + fused DVE reorder-cast: feeds from
                    # the RTL descriptor generator instead of Q7
                    ffh = ffhp.tile([N1, FCHUNK, N2], F32, name="ffh")
                    nc.sync.dma_start(out=ffh[:], in_=src)
                    nc.vector.tensor_copy(
                        ffb[:, :, lo:hi], ffh[:].rearrange("p c n -> p n c")
                    )
                    continue
                # cast inside the SWDGE DMA, then free-dim transpose
                # [p, c, n] -> [p, n, c] (1x strided-src copies, RCHUNK
                # channels at a time so single ops stay short)
                ffc = ffp32.tile([N1, FCHUNK, N2], BF16)
                nc.gpsimd.dma_start(out=ffc[:], in_=src)
                for rc in range(FCHUNK // RCHUNK):
                    r0, r1 = rc * RCHUNK, (rc + 1) * RCHUNK
                    cast(
                        ffb[:, :, lo + r0 : lo + r1],
                        ffc[:, r0:r1, :].rearrange("p c n -> p n c"),
                    )
                continue
            ff32 = ffp32.tile([N1, FCHUNK, N2], F32)
            nc.sync.dma_start(out=ff32[:], in_=src)
            if LAYOUT == "reorder":
                cast(ffb[:, :, lo:hi], ff32[:].rearrange("p c n -> p n c"))
            else:
                cast(ffb[:, lo:hi, :], ff32[:])

        if LAYOUT == "fe":
            # feats stationary: out[c, k] += ffb[:, :, n2].T @ eee[:, n2, :]
            c_ps = pscp.tile([CCHUNK, K], F32)
            for n2 in range(N2):
                nc.tensor.matmul(
                    c_ps[:], ffb[:, :, n2], eee[:, n2, :],
                    start=(n2 == 0), stop=(n2 == N2 - 1),
                )
            c_sb = csbp.tile([CCHUNK, K], F32)
            nc.vector.tensor_mul(c_sb[:], c_ps[:], rec_b[0:CCHUNK, :])
            nc.scalar.dma_start(
                out=out[cc * CCHUNK : (cc + 1) * CCHUNK, :], in_=c_sb[:]
            )
            continue

        # The LAST chunk runs as two half-width matmul groups: the first
        # half's matmuls only need the first two reorder pieces, so they
        # overlap the final pieces' load, and the closing dependency chain
        # is a 64-wide group instead of 128.
        last = cc == C // CCHUNK - 1
        groups = (
            [(0, CCHUNK // 2), (CCHUNK // 2, CCHUNK)] if last
            else [(0, CCHUNK)]
        )
        for g0, g1 in groups:
            c_ps = pscp.tile([K, g1 - g0], F32, name="c_ps")
            for n2 in range(N2):
                if LAYOUT in ("reorder", "swdge2"):
                    lhsT, rhs = eee[:, n2, :], ffb[:, n2, g0:g1]
                else:
                    lhsT, rhs = eee[:, n2, :], ffb[:, g0:g1, n2]
                nc.tensor.matmul(
                    c_ps[:], lhsT, rhs, start=(n2 == 0), stop=(n2 == N2 - 1)
                )

            # normalize: C_sb = C_ps * (1/S) per partition (= per k)
            c_sb = csbp.tile([K, g1 - g0], F32, name="c_sb")
            nc.scalar.activation(
                c_sb[:], c_ps[:], mybir.ActivationFunctionType.Copy,
                scale=rec[:],
            )
            nc.scalar.dma_start(
                out=out[:, cc * CCHUNK + g0 : cc * CCHUNK + g1], in_=c_sb[:]
            )


_NC_CACHE = {}


def _build(reps=1):
    key = (
        reps, LAYOUT, CCHUNK, FCHUNK, RCHUNK, F32_BUFS, B16_BUFS,
        CAST_ENGINES, DMA_SCRATCH, W, HW_EVERY,
    )
    if key in _NC_CACHE:
        return _NC_CACHE[key]
    nc = bacc.Bacc(
        "TRN2",
        target_bir_lowering=False,
        debug=False,
        num_devices=B,
        dynamic_dma_scratch_size=4096 if LAYOUT == "nat" else DMA_SCRATCH,
    )
    feats = nc.dram_tensor("feats", [C, N1 * N2], F32, kind="ExternalInput").ap()
    probs = nc.dram_tensor("probs", [K, N1 * N2], F32, kind="ExternalInput").ap()
    out_shape = [C, K] if LAYOUT == "fe" else [K, C]
    out = nc.dram_tensor("out", out_shape, F32, kind="ExternalOutput").ap()
    with tile.TileContext(nc) as tc:
        if LAYOUT == "nat":
            with (
                tc.tile_pool(name="ff32", bufs=NAT_F32_BUFS) as ffp32,
                tc.tile_pool(name="ff16", bufs=NAT_B16_BUFS) as ffp16,
                tc.tile_pool(name="ffb", bufs=NAT_FFB_BUFS) as ffbp,
                tc.tile_pool(name="ee", bufs=1) as eep,
                tc.tile_pool(name="small", bufs=1) as smallp,
                tc.tile_pool(name="csb", bufs=1) as csbp,
                tc.tile_pool(name="psc", bufs=1, space="PSUM") as pscp,
                tc.tile_pool(name="pse", bufs=2, space="PSUM") as psep,
                tc.tile_pool(name="pss", bufs=1, space="PSUM") as pssp,
            ):
                pools = (
                    ffp32, ffp16, ffbp, eep, smallp, csbp, pscp, psep, pssp
                )
                for _ in range(reps):
                    _body_nat(tc, pools, out, feats, probs)
        else:
            with (
                tc.tile_pool(name="ff32", bufs=F32_BUFS) as ffp32,
                tc.tile_pool(name="ff16", bufs=B16_BUFS) as ffp16,
                tc.tile_pool(name="ffh", bufs=2) as ffhp,
                tc.tile_pool(name="ee", bufs=2) as eep,
                tc.tile_pool(name="small", bufs=2) as smallp,
                tc.tile_pool(name="csb", bufs=2) as csbp,
                tc.tile_pool(name="psc", bufs=2, space="PSUM") as pscp,
                tc.tile_pool(name="pss", bufs=2, space="PSUM") as pssp,
            ):
                pools = (ffp32, ffp16, ffhp, eep, smallp, csbp, pscp, pssp)
                for _ in range(reps):
                    _body(tc, pools, out, feats, probs)
    nc.compile()
    _NC_CACHE[key] = nc
    return nc


def kernel(feats: np.ndarray, probs: np.ndarray) -> np.ndarray:
    assert feats.shape == (B, C, N1, N2) and probs.shape == (B, K, N1, N2)
    nc = _build()
    in_maps = [
        {
            "feats": np.ascontiguousarray(feats[b]).reshape(C, N1 * N2),
            "probs": np.ascontiguousarray(probs[b]).reshape(K, N1 * N2),
        }
        for b in range(B)
    ]
    res = run_bass_kernel_spmd(nc, in_maps, core_ids=list(range(B)))
    out = np.stack([res.results[b]["out"] for b in range(B)])
    if LAYOUT != "fe":  # (B, K, C) -> (B, C, K)
        out = out.transpose(0, 2, 1)
    return np.ascontiguousarray(out)[..., None].astype(np.float32)


if __name__ == "__main__":
    rng = np.random.default_rng(0)
    f = rng.standard_normal((B, C, N1, N2), dtype=np.float32)
    p = rng.standard_normal((B, K, N1, N2), dtype=np.float32)
    o = kernel(f, p)
    print("out", o.shape, o.dtype)


# revision 46
# speedup vs baseline: 1.0563x; 1.0563x over previous
"""Trainium2 Bass kernel: softmax spatial pooling (attention pooling).

Reference computation per batch b:
    attn = softmax(probs[b].reshape(19, 16384), axis=1)
    ctx  = attn @ feats[b].reshape(512, 16384).T        # (19, 512)
    out[b] = ctx.T[..., None]                           # (512, 19, 1)

Full inputs:  feats (8, 512, 128, 128) f32, probs (8, 19, 128, 128) f32.
Sharding: pure data parallel - one batch sample per NeuronCore (8 cores).

Device-side algorithm (per core):
  View n = 16384 as (n1=128, n2=128) with n1 on SBUF partitions; the PE
  contracts over n1 (partitions) and PSUM accumulates over n2.
  Softmax is unnormalized (E = exp(p)); the 1/sum normalization is applied
  once to the small per-chunk result.

  feats (32 MB fp32, the dominant cost) loads via SWDGE cast-DMA
  (fp32->bf16 in the DMA, 512B descriptors at bus rate ~22ns/desc).  That
  forces SBUF layout [n1, c, n2].  Matmul layouts:
    'fe'     feats tile is the STATIONARY operand (lhsT, strided columns),
             E is the streamed rhs (contiguous, 19 cols); out [c, k].
    'swdge2' reorder [n1,c,n2]->[n1,n2,c] on engines, E stationary,
             feats rhs contiguous; out [k, c].
    'swdge'  no reorder, feats rhs strided (4 cyc/col PE penalty).
"""

import numpy as np

import concourse.bacc as bacc
import concourse.bass as bass
import concourse.tile as tile
from concourse import mybir
from concourse.bass_utils import run_bass_kernel_spmd

B = 8          # batch == number of cores
C = 512        # feature channels
K = 19         # attention heads (probs channels)
N1 = 128       # spatial high bits -> SBUF partitions
N2 = 128       # spatial low bits  -> PSUM-accumulated matmuls

LAYOUT = "swdge2"    # 'nat' | 'fe' | 'swdge2' | 'swdge' | 'reorder' | 'direct'
CCHUNK = 128         # matmul tile width (c channels per PSUM group)
FCHUNK = 32          # channels per feats DMA piece
RCHUNK = 32          # channels per reorder-copy op (sub-slices of a piece).
                     # 16 (half-ops on both engines concurrently, DVE ops
                     # inside the ring buffer) measured neutral: 126.4us
                     # vs 124-129 here; the extra per-op overheads eat
                     # the concurrency gain.
F32_BUFS = 8   # 10 measured 132us (SBUF pressure) and 6 measured 135us;
               # 8 is the verified optimum
# Every HW_EVERY-th piece (offset 2) loads fp32 via the HWDGE sync ring
# with a fused DVE reorder-cast, relieving the Q7 SWDGE descriptor
# generator.  Measured SLOWER (135us vs 124us): the pricier HWDGE
# descriptors and DVE port-lock slivers outweigh the Q7 relief.  Disabled.
HW_EVERY = 0
B16_BUFS = 2
# Reorder-copy engines ('swdge2'/'reorder'): alternate Scalar/Vector.  A DVE
# copy locks SWDGE descriptor generation out of the shared GPSIMD/Vector
# SBUF port for its duration - reorder ops are kept small (~3.6us DVE ops)
# so the descriptor ring (DMA_SCRATCH/16 descs of drain buffer) mostly
# rides out the lockout.
CAST_ENGINES = "sv"
DMA_SCRATCH = 32768  # SWDGE descriptor-ring bytes/partition (16B/desc).
                     # 49152 (ring big enough to fully cover a DVE
                     # reorder's SWDGE lockout) measured SLOWER (136us);
                     # keep 32768.

# 'nat' layout parameters
W = 2048             # n-elements per feats DMA slab (8KB descriptors)
NAT_F32_BUFS = 4
NAT_B16_BUFS = 4
NAT_FFB_BUFS = 2

F32 = mybir.dt.float32
BF16 = mybir.dt.bfloat16


def _body_nat(tc, pools, out, feats, probs):
    """Natural-layout path: HWDGE big-descriptor fp32 loads (no SWDGE at
    all), on-chip fp32->bf16 cast, PE transpose of 128x128 blocks via
    identity matmuls, PSUM evacuation into MM-ready [n_lo, s, c] layout,
    and one persistent PSUM bank accumulating the (K, C) output over all
    128 slice-matmuls of N=C columns.

    n = 128*s + j: s = high 7 bits (slice index), j = low 7 bits (partitions
    after transpose).  E must match: E-slices [j, k] per s come from 19
    PE transposes of the exp'd probs.
    """
    nc = tc.nc
    ffp32, ffp16, ffbp, eep, smallp, csbp, pscp, psep, pssp = pools
    SLABS = (N1 * N2) // W   # feats DMA slabs
    TPS = W // 128           # 128-blocks per slab
    CB = C // 128            # channel blocks

    # identity [128, 128] bf16: iota(j - p) == 0   (iota lives on gpsimd,
    # which is otherwise idle in this layout)
    iot = smallp.tile([128, 128], mybir.dt.int32, name="iot")
    nc.gpsimd.iota(iot[:], pattern=[[1, 128]], base=0, channel_multiplier=-1)
    ident = smallp.tile([128, 128], BF16, name="ident")
    nc.gpsimd.tensor_scalar(
        ident[:], iot[:], 0, None, op0=mybir.AluOpType.is_equal
    )

    # probs (K, N1*N2) -> (N1, K, N2) on the scalar HWDGE ring
    probs_r = probs.rearrange("k (n1 n2) -> n1 k n2", n1=N1)
    ee = eep.tile([N1, K, N2], F32)
    nc.scalar.dma_start(out=ee[:], in_=probs_r)

    # E = exp(p), [n1, k, n2] bf16 (contiguous src AND dst)
    eeekn = eep.tile([N1, K, N2], BF16)
    nc.scalar.activation(eeekn[:], ee[:], mybir.ActivationFunctionType.Exp)

    # partials[n1, k] = sum_n2 E[k, n1, n2] (contiguous reduce)
    partials = smallp.tile([N1, K, 1], F32)
    nc.vector.reduce_sum(out=partials[:], in_=eeekn[:], axis=mybir.AxisListType.X)
    ones = smallp.tile([N1, 1], F32)
    nc.vector.memset(ones[:], 1.0)
    s_ps = pssp.tile([K, 1], F32)
    nc.tensor.matmul(s_ps[:], partials[:, :, 0], ones[:], start=True, stop=True)
    rec = smallp.tile([K, 1], F32)
    nc.vector.reciprocal(rec[:], s_ps[:])

    # E-transpose: eee2[j, s, k] = E[k, 128*s + j] via 19 PE transposes
    eee2 = eep.tile([N2, N1, K], BF16)
    for k in range(K):
        psE = psep.tile([128, N1], F32)
        nc.tensor.matmul(psE[:], eeekn[:, k, :], ident[:], start=True, stop=True)
        nc.vector.tensor_copy(eee2[:, :, k], psE[:])

    # feats loads: [c-block 128 rows, W-column slab] fp32, 8KB descriptors
    feats_cb = feats.rearrange("(cb p) n -> cb p n", p=128)
    big_ps = pscp.tile([K, C], F32)
    cast_i = 0
    evac_i = 0
    for slab in range(SLABS):
        ffb = ffbp.tile([128, TPS, C], BF16)
        for cb in range(CB):
            f32 = ffp32.tile([128, W], F32)
            nc.sync.dma_start(
                out=f32[:], in_=feats_cb[cb, :, slab * W : (slab + 1) * W]
            )
            b16 = ffp16.tile([128, W], BF16)
            if cast_i % 2 == 0:
                nc.vector.tensor_copy(b16[:], f32[:])
            else:
                nc.scalar.copy(b16[:], f32[:])
            cast_i += 1
            # PE transposes, 4 blocks per PSUM bank, then one evacuation
            for t0 in range(0, TPS, 4):
                psT = psep.tile([128, 512], F32, tag="psT")
                for u in range(4):
                    nc.tensor.matmul(
                        psT[:, u * 128 : (u + 1) * 128],
                        b16[:, (t0 + u) * 128 : (t0 + u + 1) * 128],
                        ident[:],
                        start=True, stop=True,
                    )
                dst = ffb[:, t0 : t0 + 4, cb * 128 : (cb + 1) * 128]
                src = psT[:].rearrange("p (t c) -> p t c", t=4)
                if evac_i % 2 == 0:
                    nc.scalar.copy(dst, src)
                else:
                    nc.vector.tensor_copy(dst, src)
                evac_i += 1
        # slice-matmuls for this slab, all accumulating into big_ps
        for t in range(TPS):
            s = slab * TPS + t
            nc.tensor.matmul(
                big_ps[:], eee2[:, s, :], ffb[:, t, :],
                start=(s == 0), stop=(s == N1 - 1),
            )

    # normalize and store (K, C)
    c_sb = csbp.tile([K, C], F32)
    nc.scalar.activation(
        c_sb[:], big_ps[:], mybir.ActivationFunctionType.Copy, scale=rec[:]
    )
    nc.scalar.dma_start(out=out[:], in_=c_sb[:])


def _body(tc, pools, out, feats, probs):
    nc = tc.nc
    ffp32, ffp16, ffhp, eep, smallp, csbp, pscp, pssp = pools

    # probs (K, N1*N2) -> (N1, K, N2): 512B contiguous runs per (n1, k).
    # On the sync (SP) HWDGE ring: feats go via SWDGE so SP is otherwise
    # idle, and the Scalar sequencer (EXP + reorders + normalizes) is the
    # busiest - keep DMA trigger dispatch off it.
    probs_r = probs.rearrange("k (n1 n2) -> n1 k n2", n1=N1)
    ee = eep.tile([N1, K, N2], F32)
    nc.sync.dma_start(out=ee[:], in_=probs_r)

    # E = exp(p) stored [n1, n2, k] so each matmul slice [:, n2, :] is
    # contiguous.  The strided rearrange goes on the SOURCE side: engines
    # stream ~1 elem/cycle with a contiguous dst even when src is strided,
    # but a strided dst costs ~5 cycles/elem.
    eee = eep.tile([N1, N2, K], BF16)
    nc.scalar.activation(
        eee[:], ee[:].rearrange("p k n -> p n k"),
        mybir.ActivationFunctionType.Exp,
    )

    # partials[n1, k] = sum_n2 E[k, n1, n2]
    partials = smallp.tile([N1, K, 1], F32)
    nc.vector.reduce_sum(
        out=partials[:], in_=eee[:].rearrange("p n k -> p k n"),
        axis=mybir.AxisListType.X,
    )

    ones = smallp.tile([N1, 1], F32)
    nc.vector.memset(ones[:], 1.0)
    if LAYOUT == "fe":
        # S as a row vector (1, K); reciprocal; broadcast down 128
        # partitions via a rank-1 matmul so DVE can multiply along free dim.
        s_ps = pssp.tile([1, K], F32)
        nc.tensor.matmul(s_ps[:], ones[:], partials[:, :, 0], start=True, stop=True)
        rec_t = smallp.tile([1, K], F32)
        nc.vector.reciprocal(rec_t[:], s_ps[:])
        ones_row = smallp.tile([1, N1], F32)
        nc.vector.memset(ones_row[:], 1.0)
        bc_ps = pssp.tile([N1, K], F32, tag="bc")
        nc.tensor.matmul(bc_ps[:], ones_row[:], rec_t[:], start=True, stop=True)
        rec_b = smallp.tile([N1, K], F32)
        nc.scalar.copy(rec_b[:], bc_ps[:])
        rec = None
    else:
        # S[k] = sum_n1 partials[n1, k]; rec = 1/S per partition (k rows)
        s_ps = pssp.tile([K, 1], F32)
        nc.tensor.matmul(s_ps[:], partials[:, :, 0], ones[:], start=True, stop=True)
        rec = smallp.tile([K, 1], F32)
        nc.vector.reciprocal(rec[:], s_ps[:])
        rec_b = None

    # feats (C, N1*N2) -> (N1, C, N2) view; DMA pieces of FCHUNK channels
    feats_r = feats.rearrange("c (n1 n2) -> n1 c n2", n1=N1)
    n_f_per_c = CCHUNK // FCHUNK
    cast_i = 0

    def cast(dst_view, src_view):
        nonlocal cast_i
        eng = CAST_ENGINES[cast_i % len(CAST_ENGINES)]
        cast_i += 1
        if eng == "v":
            nc.vector.tensor_copy(dst_view, src_view)
        elif eng == "s":
            nc.scalar.copy(dst_view, src_view)
        else:
            nc.gpsimd.tensor_copy(dst_view, src_view)

    for cc in range(C // CCHUNK):
        if LAYOUT in ("reorder", "swdge2"):
            ffb = ffp16.tile([N1, N2, CCHUNK], BF16)
        else:
            ffb = ffp16.tile([N1, CCHUNK, N2], BF16)
        for fc in range(n_f_per_c):
            c0 = cc * CCHUNK + fc * FCHUNK
            lo, hi = fc * FCHUNK, (fc + 1) * FCHUNK
            src = feats_r[:, c0 : c0 + FCHUNK, :]
            if LAYOUT in ("swdge", "fe"):
                # fp32->bf16 cast inside the SWDGE DMA; one descriptor per
                # (n1, c) 512B run
                nc.gpsimd.dma_start(out=ffb[:, lo:hi, :], in_=src)
                continue
            if LAYOUT == "swdge2":
                gfc = cc * n_f_per_c + fc
                if HW_EVERY and gfc % HW_EVERY == 2:
                    # HWDGE fp32 load + fused DVE reorder-cast: feeds from
                    # the RTL descriptor generator instead of Q7
                    ffh = ffhp.tile([N1, FCHUNK, N2], F32, name="ffh")
                    nc.sync.dma_start(out=ffh[:], in_=src)
                    nc.vector.tensor_copy(
                        ffb[:, :, lo:hi], ffh[:].rearrange("p c n -> p n c")
                    )
                    continue
                # cast inside the SWDGE DMA, then free-dim transpose
                # [p, c, n] -> [p, n, c] (1x strided-src copies, RCHUNK
                # channels at a time so single ops stay short)
                ffc = ffp32.tile([N1, FCHUNK, N2], BF16)
                nc.gpsimd.dma_start(out=ffc[:], in_=src)
                for rc in range(FCHUNK // RCHUNK):
                    r0, r1 = rc * RCHUNK, (rc + 1) * RCHUNK
                    cast(
                        ffb[:, :, lo + r0 : lo + r1],
                        ffc[:, r0:r1, :].rearrange("p c n -> p n c"),
                    )
                continue
            ff32 = ffp32.tile([N1, FCHUNK, N2], F32)
            nc.sync.dma_start(out=ff32[:], in_=src)
            if LAYOUT == "reorder":
                cast(ffb[:, :, lo:hi], ff32[:].rearrange("p c n -> p n c"))
            else:
                cast(ffb[:, lo:hi, :], ff32[:])

        if LAYOUT == "fe":
            # feats stationary: out[c, k] += ffb[:, :, n2].T @ eee[:, n2, :]
            c_ps = pscp.tile([CCHUNK, K], F32)
            for n2 in range(N2):
                nc.tensor.matmul(
                    c_ps[:], ffb[:, :, n2], eee[:, n2, :],
                    start=(n2 == 0), stop=(n2 == N2 - 1),
                )
            c_sb = csbp.tile([CCHUNK, K], F32)
            nc.vector.tensor_mul(c_sb[:], c_ps[:], rec_b[0:CCHUNK, :])
            nc.scalar.dma_start(
                out=out[cc * CCHUNK : (cc + 1) * CCHUNK, :], in_=c_sb[:]
            )
            continue

        # The LAST chunk runs as two half-width matmul groups: the first
        # half's matmuls only need the first two reorder pieces, so they
        # overlap the final pieces' load, and the closing dependency chain
        # is a 64-wide group instead of 128.
        last = cc == C // CCHUNK - 1
        groups = (
            [(0, CCHUNK // 2), (CCHUNK // 2, CCHUNK)] if last
            else [(0, CCHUNK)]
        )
        for g0, g1 in groups:
            c_ps = pscp.tile([K, g1 - g0], F32, name="c_ps")
            for n2 in range(N2):
                if LAYOUT in ("reorder", "swdge2"):
                    lhsT, rhs = eee[:, n2, :], ffb[:, n2, g0:g1]
                else:
                    lhsT, rhs = eee[:, n2, :], ffb[:, g0:g1, n2]
                nc.tensor.matmul(
                    c_ps[:], lhsT, rhs, start=(n2 == 0), stop=(n2 == N2 - 1)
                )

            # normalize: C_sb = C_ps * (1/S) per partition (= per k)
            c_sb = csbp.tile([K, g1 - g0], F32, name="c_sb")
            nc.scalar.activation(
                c_sb[:], c_ps[:], mybir.ActivationFunctionType.Copy,
                scale=rec[:],
            )
            # store via the idle SP ring, not the busy Scalar sequencer
            nc.sync.dma_start(
                out=out[:, cc * CCHUNK + g0 : cc * CCHUNK + g1], in_=c_sb[:]
            )


_NC_CACHE = {}


def _build(reps=1):
    key = (
        reps, LAYOUT, CCHUNK, FCHUNK, RCHUNK, F32_BUFS, B16_BUFS,
        CAST_ENGINES, DMA_SCRATCH, W, HW_EVERY,
    )
    if key in _NC_CACHE:
        return _NC_CACHE[key]
    nc = bacc.Bacc(
        "TRN2",
        target_bir_lowering=False,
        debug=False,
        num_devices=B,
        dynamic_dma_scratch_size=4096 if LAYOUT == "nat" else DMA_SCRATCH,
    )
    feats = nc.dram_tensor("feats", [C, N1 * N2], F32, kind="ExternalInput").ap()
    probs = nc.dram_tensor("probs", [K, N1 * N2], F32, kind="ExternalInput").ap()
    out_shape = [C, K] if LAYOUT == "fe" else [K, C]
    out = nc.dram_tensor("out", out_shape, F32, kind="ExternalOutput").ap()
    with tile.TileContext(nc) as tc:
        if LAYOUT == "nat":
            with (
                tc.tile_pool(name="ff32", bufs=NAT_F32_BUFS) as ffp32,
                tc.tile_pool(name="ff16", bufs=NAT_B16_BUFS) as ffp16,
                tc.tile_pool(name="ffb", bufs=NAT_FFB_BUFS) as ffbp,
                tc.tile_pool(name="ee", bufs=1) as eep,
                tc.tile_pool(name="small", bufs=1) as smallp,
                tc.tile_pool(name="csb", bufs=1) as csbp,
                tc.tile_pool(name="psc", bufs=1, space="PSUM") as pscp,
                tc.tile_pool(name="pse", bufs=2, space="PSUM") as psep,
                tc.tile_pool(name="pss", bufs=1, space="PSUM") as pssp,
            ):
                pools = (
                    ffp32, ffp16, ffbp, eep, smallp, csbp, pscp, psep, pssp
                )
                for _ in range(reps):
                    _body_nat(tc, pools, out, feats, probs)
        else:
            with (
                tc.tile_pool(name="ff32", bufs=F32_BUFS) as ffp32,
                tc.tile_pool(name="ff16", bufs=B16_BUFS) as ffp16,
                tc.tile_pool(name="ffh", bufs=2) as ffhp,
                tc.tile_pool(name="ee", bufs=2) as eep,
                tc.tile_pool(name="small", bufs=2) as smallp,
                tc.tile_pool(name="csb", bufs=2) as csbp,
                tc.tile_pool(name="psc", bufs=2, space="PSUM") as pscp,
                tc.tile_pool(name="pss", bufs=2, space="PSUM") as pssp,
            ):
                pools = (ffp32, ffp16, ffhp, eep, smallp, csbp, pscp, pssp)
                for _ in range(reps):
                    _body(tc, pools, out, feats, probs)
    nc.compile()
    _NC_CACHE[key] = nc
    return nc


def kernel(feats: np.ndarray, probs: np.ndarray) -> np.ndarray:
    assert feats.shape == (B, C, N1, N2) and probs.shape == (B, K, N1, N2)
    nc = _build()
    in_maps = [
        {
            "feats": np.ascontiguousarray(feats[b]).reshape(C, N1 * N2),
            "probs": np.ascontiguousarray(probs[b]).reshape(K, N1 * N2),
        }
        for b in range(B)
    ]
    res = run_bass_kernel_spmd(nc, in_maps, core_ids=list(range(B)))
    out = np.stack([res.results[b]["out"] for b in range(B)])
    if LAYOUT != "fe":  # (B, K, C) -> (B, C, K)
        out = out.transpose(0, 2, 1)
    return np.ascontiguousarray(out)[..., None].astype(np.float32)


if __name__ == "__main__":
    rng = np.random.default_rng(0)
    f = rng.standard_normal((B, C, N1, N2), dtype=np.float32)
    p = rng.standard_normal((B, K, N1, N2), dtype=np.float32)
    o = kernel(f, p)
    print("out", o.shape, o.dtype)
